# revision 11
# baseline (speedup 1.0000x reference)
"""Trainium2 Bass kernel for nn_LGnet (LSTM + memory attention recurrence).

Sharding: data-parallel over batch, B=256 -> 32 rows per core across 8 cores.

Design (all matmuls 16-bit, minimal critical chain):
  - z/zp gating streams are pure input preprocessing -> folded on HOST.
  - logits computed DIRECTLY: logits^T = MW @ H + logz[t], where
    MW = 0.5*memory@(Wq3@W_fc) (host-folded) and logz[t] = RZ.T@z + RZP.T@zp
    + mbq is precomputed on-device (matmuls; slices interleaved into the
    first steps) and injected into PSUM via identity-rhs matmuls.
  - softmax denominator via broadcast-sums trick (all-ones stationary ->
    per-partition-replicated sums), one reciprocal, one multiply.
  - LSTM state kept as H=2h, D=2c; i/f/o rows of weights+bias host-scaled
    by 0.5 so the whole gate nonlinearity is ONE tanh table (exp+tanh share
    an ACT table set; sigmoid does not), pointwise is 4 fused
    scalar_tensor_tensor ops:  m1=(Yf+1)*D; m2=(Yi+1)*Yg; D'=0.5*m1+m2;
    H'=(Yo+1)*tanh(0.5*D').
  - W_hh matmul stream overlaps the softmax scalar chain; sums/gd issued
    early so gdn is ready before the W_ih matmuls; o-gate chunks go to a
    separate PSUM bank and are computed last so the c-chain overlaps the
    remaining MMs.
  - gate bias injected into PSUM by [16,128]x[16,*] matmuls.
  - fp16 everywhere except exp outputs / reciprocal (bf16 for range).
"""
import os
import numpy as np
from contextlib import ExitStack

B, T, F, H, O, M = 256, 100, 128, 512, 128, 512
T = int(os.environ.get("LG_T", str(T)))   # debug override; harness uses 100
assert T % 4 == 0
NC = 8
BB = B // NC          # 32 batch rows per core
TB = T * BB           # (t, b) packed columns
NSL = TB // 128       # 128-col slices (= T/4 tgroups)

_built = None


def _build():
    import concourse.bass as bass
    import concourse.tile as tile
    from concourse import bacc, mybir

    f32 = mybir.dt.float32
    bf16 = mybir.dt.bfloat16
    f16 = mybir.dt.float16
    AF = mybir.ActivationFunctionType
    ALU = mybir.AluOpType
    nc = bacc.Bacc("TRN2", target_bir_lowering=False, debug=False, num_devices=NC)

    # ---- DRAM tensors (per-core data fed via in_maps) ----
    z_d = nc.dram_tensor("z", [F, TB], f16, kind="ExternalInput").ap()
    zp_d = nc.dram_tensor("zp", [F, TB], f16, kind="ExternalInput").ap()
    mw_d = nc.dram_tensor("mw", [128, 16 * 128], f16, kind="ExternalInput").ap()
    whh_d = nc.dram_tensor("whh", [128, 64 * 128], f16, kind="ExternalInput").ap()
    wih_d = nc.dram_tensor("wih", [128, 16 * 128], f16, kind="ExternalInput").ap()
    membf_d = nc.dram_tensor("membf", [128, 512], bf16, kind="ExternalInput").ap()
    rz_d = nc.dram_tensor("rz", [128, 512], f16, kind="ExternalInput").ap()
    rzp_d = nc.dram_tensor("rzp", [128, 512], f16, kind="ExternalInput").ap()
    wfct_d = nc.dram_tensor("wfct", [128, 512], f16, kind="ExternalInput").ap()
    bias16_d = nc.dram_tensor("bias16", [16, 128], f16, kind="ExternalInput").ap()
    ind_d = nc.dram_tensor("ind", [16, 512], f16, kind="ExternalInput").ap()
    i32sel_d = nc.dram_tensor("i32sel", [128, 128], f16, kind="ExternalInput").ap()
    mbqb_d = nc.dram_tensor("mbqb", [128, 512], f16, kind="ExternalInput").ap()
    bfc1_d = nc.dram_tensor("bfc1", [1, 128], f16, kind="ExternalInput").ap()
    o_d = nc.dram_tensor("o", [O, BB], f32, kind="ExternalOutput").ap()

    with tile.TileContext(nc) as tc, ExitStack() as ctx:
        wpool = ctx.enter_context(tc.tile_pool(name="wpool", bufs=1))
        stp = ctx.enter_context(tc.tile_pool(name="stp", bufs=2))
        state = ctx.enter_context(tc.tile_pool(name="state", bufs=2))
        pers = ctx.enter_context(tc.tile_pool(name="pers", bufs=1))
        attn_ps = ctx.enter_context(tc.tile_pool(name="attn_ps", bufs=2, space="PSUM"))
        gi_ps = ctx.enter_context(tc.tile_pool(name="gi_ps", bufs=2, space="PSUM"))
        go_ps = ctx.enter_context(tc.tile_pool(name="go_ps", bufs=2, space="PSUM"))
        pre_ps = ctx.enter_context(tc.tile_pool(name="pre_ps", bufs=2, space="PSUM"))

        # ---- static weights / inputs into SBUF ----
        def wload(name, shape, dt_, src):
            t_ = wpool.tile(shape, dt_, tag=name)
            nc.sync.dma_start(t_[:], src[:])
            return t_
        Z = wload("Z", [128, TB], f16, z_d)
        ZP = wload("ZP", [128, TB], f16, zp_d)
        MW = wload("MW", [128, 16 * 128], f16, mw_d)
        WHH = wload("WHH", [128, 64 * 128], f16, whh_d)
        WIH = wload("WIH", [128, 16 * 128], f16, wih_d)
        MEMBF = wload("MEMBF", [128, 512], bf16, membf_d)
        RZ = wload("RZ", [128, 512], f16, rz_d)
        RZP = wload("RZP", [128, 512], f16, rzp_d)
        WFCT = wload("WFCT", [128, 512], f16, wfct_d)
        BIAS16 = wload("BIAS16", [16, 128], f16, bias16_d)
        IND = wload("IND", [16, 512], f16, ind_d)
        I32S = wload("I32S", [128, 128], f16, i32sel_d)
        MBQB = wload("MBQB", [128, 512], f16, mbqb_d)
        BFC1 = wload("BFC1", [1, 128], f16, bfc1_d)
        ONESB = wpool.tile([128, 128], bf16, tag="ONESB")
        nc.vector.memset(ONESB[:], 1.0)
        ONES32 = wpool.tile([1, 32], f16, tag="ONES32")
        nc.vector.memset(ONES32[:], 1.0)

        logz = wpool.tile([128, NSL * 512], f16, tag="logz")

        HT = pers.tile([128, 128], f16, tag="HT")
        Dt = pers.tile([128, 128], f16, tag="Dt")
        nc.vector.memset(HT[:], 0.0)
        nc.vector.memset(Dt[:], 0.0)

        # logz slice s: [tb-part, m] = Z_s.T @ RZ + ZP_s.T @ RZP, + mbq via
        # a Pool-engine add during the PSUM->SBUF copy (Pool is otherwise idle).
        def logz_slice(s):
            pp = pre_ps.tile([128, 512], f32, tag="pp")
            nc.tensor.matmul(pp[:], lhsT=Z[:, 128 * s:128 * (s + 1)],
                             rhs=RZ[:], start=True, stop=False)
            nc.tensor.matmul(pp[:], lhsT=ZP[:, 128 * s:128 * (s + 1)],
                             rhs=RZP[:], start=False, stop=True)
            # (GPSIMD cannot access PSUM; V is idle at step start anyway)
            nc.vector.tensor_tensor(logz[:, 512 * s:512 * (s + 1)], pp[:],
                                    MBQB[:], ALU.add)

        with nc.named_scope("precompute"):
            for s in range(min(4, NSL)):
                logz_slice(s)

        # ---- recurrence ----
        for t in range(T):
            tg, tl = t // 4, t % 4
            with nc.named_scope(f"step{t}" if t % 10 == 0 else "step"):
                if 4 + t < NSL:              # stream remaining logz slices
                    logz_slice(4 + t)        # (ready long before step 4*(4+t))
                pa = attn_ps.tile([128, 512], f32, tag="pa")
                pgi = gi_ps.tile([128, 384], f32, tag="pgi")
                pgo = go_ps.tile([128, 128], f32, tag="pgo")

                # gate bias into psum (constants only; runs during prev tail)
                nc.tensor.matmul(pgi[:], lhsT=BIAS16[:], rhs=IND[:, 0:384],
                                 start=True, stop=False)
                nc.tensor.matmul(pgo[:], lhsT=BIAS16[:], rhs=IND[:, 384:512],
                                 start=True, stop=False)
                # logits: one sequential accumulation group per 32-col slice
                for j in range(4):
                    nc.tensor.matmul(pa[:, 32 * j:32 * (j + 1)],
                                     lhsT=logz[:, 512 * tg + 128 * j:512 * tg + 128 * (j + 1)],
                                     rhs=I32S[:, 32 * tl:32 * tl + 32],
                                     start=True, stop=False)
                    for k in range(4):
                        nc.tensor.matmul(pa[:, 32 * j:32 * (j + 1)],
                                         lhsT=MW[:, 128 * (4 * j + k):128 * (4 * j + k + 1)],
                                         rhs=HT[:, 32 * k:32 * k + 32],
                                         start=False, stop=(k == 3))
                # e^T = exp(logits^T)  [128 m-part, (chunk, b)]
                ET = stp.tile([128, 128], bf16, tag="ET")
                nc.scalar.activation(ET[:], pa[:, 0:128], AF.Exp)

                def whh_block(glo, ghi):
                    for g in range(glo, ghi):
                        dst = pgi if g < 12 else pgo
                        off = 32 * g if g < 12 else 32 * (g - 12)
                        for k in range(4):
                            nc.tensor.matmul(dst[:, off:off + 32],
                                             lhsT=WHH[:, 128 * (4 * g + k):128 * (4 * g + k + 1)],
                                             rhs=HT[:, 32 * k:32 * k + 32],
                                             start=False, stop=False)

                # all W_hh MMs for i,f,g chunks BEFORE sums/gd: the [sums, gd]
                # batch then sits between two wait boundaries so its semaphore
                # increment fires right at gd completion (early recip signal).
                whh_block(0, 12)      # 48 MMs (exp runs concurrently)
                # high priority: the scheduler orders by (readiness, priority);
                # without this it packs all dep-free whh MMs first and the
                # sums/gd completion signal (-> recip -> gdn -> ih) fires only
                # at the end of the whole burst.
                with tc.high_priority(offset=150):
                    # sums bcast over partitions: pa[:,128:160] = sum_m e
                    for c in range(4):
                        nc.tensor.matmul(pa[:, 128:160], lhsT=ONESB[:],
                                         rhs=ET[:, 32 * c:32 * c + 32],
                                         start=(c == 0), stop=(c == 3))
                    # gd^T = memory^T-chunks @ e^T
                    for j in range(4):
                        nc.tensor.matmul(pa[:, 160:192],
                                         lhsT=MEMBF[:, 128 * j:128 * (j + 1)],
                                         rhs=ET[:, 32 * j:32 * j + 32],
                                         start=(j == 0), stop=(j == 3))
                    recipB = stp.tile([128, 32], f32, tag="recipB")
                    with nc.allow_low_precision(reason="softmax recip approx ok"):
                        nc.vector.reciprocal_approx_fast(recipB[:], pa[:, 128:160])
                    GDN = stp.tile([128, 32], f16, tag="GDN")
                    nc.vector.tensor_tensor(GDN[:], pa[:, 160:192], recipB[:],
                                            ALU.mult)
                    # W_ih @ gdn for i, f, g chunks
                    for g in range(12):
                        nc.tensor.matmul(pgi[:, 32 * g:32 * (g + 1)],
                                         lhsT=WIH[:, 128 * g:128 * (g + 1)],
                                         rhs=GDN[:], start=False, stop=(g == 11))
                whh_block(12, 16)     # o chunks while tanh/c-chain run
                for g in range(12, 16):
                    nc.tensor.matmul(pgo[:, 32 * (g - 12):32 * (g - 11)],
                                     lhsT=WIH[:, 128 * g:128 * (g + 1)],
                                     rhs=GDN[:], start=False, stop=(g == 15))

                # pointwise: Y = tanh(gates_scaled), fp16
                Yifg = stp.tile([128, 384], f16, tag="Yifg")
                nc.scalar.activation(Yifg[:], pgi[:], AF.Tanh)
                Yo = stp.tile([128, 128], f16, tag="Yo")
                nc.scalar.activation(Yo[:], pgo[:], AF.Tanh)
                m1 = stp.tile([128, 128], f16, tag="m1")
                nc.vector.scalar_tensor_tensor(m1[:], Yifg[:, 128:256], 1.0, Dt[:],
                                               ALU.add, ALU.mult)
                m2 = stp.tile([128, 128], f16, tag="m2")
                nc.vector.scalar_tensor_tensor(m2[:], Yifg[:, 0:128], 1.0,
                                               Yifg[:, 256:384], ALU.add, ALU.mult)
                Dn = state.tile([128, 128], f16, tag="Dn")
                nc.vector.scalar_tensor_tensor(Dn[:], m1[:], 0.5, m2[:],
                                               ALU.mult, ALU.add)
                TC = stp.tile([128, 128], f16, tag="TC")
                nc.scalar.activation(TC[:], Dn[:], AF.Tanh, scale=0.5)
                Hn = state.tile([128, 128], f16, tag="Hn")
                nc.vector.scalar_tensor_tensor(Hn[:], Yo[:], 1.0, TC[:],
                                               ALU.add, ALU.mult)
                HT, Dt = Hn, Dn

        # ---- final output: out^T = 0.5*W_fc @ H + b_fc ----
        with nc.named_scope("final"):
            pf = attn_ps.tile([128, 512], f32, tag="pa")
            nc.tensor.matmul(pf[:, 0:32], lhsT=BFC1[:], rhs=ONES32[:],
                             start=True, stop=False)
            for k in range(4):
                nc.tensor.matmul(pf[:, 0:32], lhsT=WFCT[:, 128 * k:128 * (k + 1)],
                                 rhs=HT[:, 32 * k:32 * k + 32],
                                 start=False, stop=(k == 3))
            outt = stp.tile([O, BB], f32, tag="outt")
            nc.scalar.activation(outt[:], pf[:, 0:32], AF.Identity)
            nc.sync.dma_start(o_d[:], outt[:])

    nc.compile()
    return nc


def _prep_host(inputs):
    """Host-side: fold weights, compute z/zp gating streams, build in_maps."""
    f16 = np.float16
    import ml_dtypes
    b16 = ml_dtypes.bfloat16
    inp = {k: np.asarray(v, np.float32) for k, v in inputs.items()}
    mem = inp["memory"]
    Wq = inp["W_q"]
    Wq1, Wq2, Wq3 = Wq[:, :F], Wq[:, F:2 * F], Wq[:, 2 * F:]

    # gate row scaling: 0.5 for i,f,o (tanh trick), 1.0 for g; W_hh also *0.5 (H=2h)
    r = np.full((4 * H, 1), 0.5, np.float32)
    r[2 * H:3 * H] = 1.0
    WIHs = r * inp["W_ih"]
    WHHs = r * inp["W_hh"] * 0.5
    biass = r[:, 0] * (inp["b_ih"] + inp["b_hh"])

    wih = np.empty((128, 16 * 128), np.float32)
    for g in range(16):
        wih[:, 128 * g:128 * (g + 1)] = WIHs[128 * g:128 * (g + 1), :].T
    whh = np.empty((128, 64 * 128), np.float32)
    for g in range(16):
        for k in range(4):
            whh[:, 128 * (4 * g + k):128 * (4 * g + k + 1)] = \
                WHHs[128 * g:128 * (g + 1), 128 * k:128 * (k + 1)].T
    MWmat = 0.5 * (mem @ (Wq3 @ inp["W_fc"]))          # [M, H]
    mw = np.empty((128, 16 * 128), np.float32)
    for j in range(4):
        for k in range(4):
            mw[:, 128 * (4 * j + k):128 * (4 * j + k + 1)] = \
                MWmat[128 * j:128 * (j + 1), 128 * k:128 * (k + 1)].T
    membf = np.empty((128, 512), np.float32)
    for j in range(4):
        membf[:, 128 * j:128 * (j + 1)] = mem[128 * j:128 * (j + 1), :]
    wfct = np.empty((128, 512), np.float32)
    WFCs = (0.5 * inp["W_fc"]).T                       # [H, O]
    for k in range(4):
        wfct[:, 128 * k:128 * (k + 1)] = WFCs[128 * k:128 * (k + 1), :]

    bias16 = biass.reshape(16, 128)
    ind = np.zeros((16, 512), np.float32)
    for g in range(16):
        ind[g, 32 * g:32 * (g + 1)] = 1.0
    i32sel = np.eye(128, dtype=f16)
    mbq = mem @ (inp["b_q"] + Wq3 @ inp["b_fc"])       # [M]
    mbqb = np.broadcast_to(mbq[None, :], (128, 512)).copy()
    bfc1 = inp["b_fc"][None, :]

    shared = dict(
        mw=mw.astype(f16), whh=whh.astype(f16), wih=wih.astype(f16),
        membf=membf.astype(b16), rz=(mem @ Wq1).T.astype(f16),
        rzp=(mem @ Wq2).T.astype(f16), wfct=wfct.astype(f16),
        bias16=bias16.astype(f16), ind=ind.astype(f16), i32sel=i32sel,
        mbqb=mbqb.astype(f16), bfc1=bfc1.astype(f16),
    )

    # z/zp gating streams on host (input-only elementwise preprocessing)
    x = inp["input"]                                   # [B, 6, 100, F]
    X, Xl, Mask, Delta, Xlb, Dltb = (x[:, i, :T] for i in range(6))
    Xm = inp["X_mean"][None, :T, :]                    # [1, T, F]
    dgz = np.diag(inp["W_gz"])[None, None, :]
    bgz = inp["b_gz"][None, None, :]
    dgzp = np.diag(inp["W_gzp"])[None, None, :]
    bgzp = inp["b_gzp"][None, None, :]
    dz = np.minimum(np.exp(-dgz * Delta - bgz), 1.0)
    dzp = np.minimum(np.exp(-dgzp * Dltb - bgzp), 1.0)
    zfull = Mask * X + (1 - Mask) * (dz * Xl + (1 - dz) * Xm)      # [B, T, F]
    zpfull = Mask * X + (1 - Mask) * (dzp * Xlb + (1 - dzp) * Xm)

    in_maps = []
    for core in range(NC):
        b0 = core * BB
        m_ = dict(shared)
        # [BB, T, F] -> [F, T*BB]
        m_["z"] = np.ascontiguousarray(
            np.transpose(zfull[b0:b0 + BB], (2, 1, 0)).reshape(F, TB)).astype(f16)
        m_["zp"] = np.ascontiguousarray(
            np.transpose(zpfull[b0:b0 + BB], (2, 1, 0)).reshape(F, TB)).astype(f16)
        in_maps.append(m_)
    return in_maps


def kernel(**inputs):
    global _built
    from concourse import bass_utils
    if _built is None:
        _built = _build()
    in_maps = _prep_host(inputs)
    res = bass_utils.run_bass_kernel_spmd(_built, in_maps, core_ids=list(range(NC)))
    out = np.empty((B, 1, O), np.float32)
    for core in range(NC):
        out[core * BB:(core + 1) * BB, 0, :] = res.results[core]["o"].T
    return out


# revision 12
# speedup vs baseline: 1.1953x; 1.1953x over previous
"""Trainium2 Bass kernel for nn_LGnet (LSTM + memory attention recurrence).

Sharding: data-parallel over batch, B=256 -> 32 rows per core across 8 cores.

Design (all matmuls 16-bit, minimal critical chain):
  - z/zp gating streams are pure input preprocessing -> folded on HOST.
  - logits computed DIRECTLY: logits^T = MW @ H + logz[t], where
    MW = 0.5*memory@(Wq3@W_fc) (host-folded) and logz[t] = RZ.T@z + RZP.T@zp
    + mbq is precomputed on-device (matmuls; slices interleaved into the
    first steps) and injected into PSUM via identity-rhs matmuls.
  - softmax denominator via broadcast-sums trick (all-ones stationary ->
    per-partition-replicated sums), one reciprocal, one multiply.
  - LSTM state kept as H=2h, D=2c; i/f/o rows of weights+bias host-scaled
    by 0.5 so the whole gate nonlinearity is ONE tanh table (exp+tanh share
    an ACT table set; sigmoid does not), pointwise is 4 fused
    scalar_tensor_tensor ops:  m1=(Yf+1)*D; m2=(Yi+1)*Yg; D'=0.5*m1+m2;
    H'=(Yo+1)*tanh(0.5*D').
  - W_hh matmul stream overlaps the softmax scalar chain; sums/gd issued
    early so gdn is ready before the W_ih matmuls; o-gate chunks go to a
    separate PSUM bank and are computed last so the c-chain overlaps the
    remaining MMs.
  - gate bias injected into PSUM by [16,128]x[16,*] matmuls.
  - fp16 everywhere except exp outputs / reciprocal (bf16 for range).
"""
import os
import numpy as np
from contextlib import ExitStack

B, T, F, H, O, M = 256, 100, 128, 512, 128, 512
T = int(os.environ.get("LG_T", str(T)))   # debug override; harness uses 100
assert T % 4 == 0
NC = 8
BB = B // NC          # 32 batch rows per core
TB = T * BB           # (t, b) packed columns
NSL = TB // 128       # 128-col slices (= T/4 tgroups)

_built = None


def _build():
    import concourse.bass as bass
    import concourse.tile as tile
    from concourse import bacc, mybir

    f32 = mybir.dt.float32
    bf16 = mybir.dt.bfloat16
    f16 = mybir.dt.float16
    AF = mybir.ActivationFunctionType
    ALU = mybir.AluOpType
    nc = bacc.Bacc("TRN2", target_bir_lowering=False, debug=False, num_devices=NC)

    # ---- DRAM tensors (per-core data fed via in_maps) ----
    z_d = nc.dram_tensor("z", [F, TB], f16, kind="ExternalInput").ap()
    zp_d = nc.dram_tensor("zp", [F, TB], f16, kind="ExternalInput").ap()
    mw_d = nc.dram_tensor("mw", [128, 16 * 128], f16, kind="ExternalInput").ap()
    whh_d = nc.dram_tensor("whh", [128, 64 * 128], f16, kind="ExternalInput").ap()
    wih_d = nc.dram_tensor("wih", [128, 16 * 128], f16, kind="ExternalInput").ap()
    membf_d = nc.dram_tensor("membf", [128, 512], bf16, kind="ExternalInput").ap()
    rz_d = nc.dram_tensor("rz", [128, 512], f16, kind="ExternalInput").ap()
    rzp_d = nc.dram_tensor("rzp", [128, 512], f16, kind="ExternalInput").ap()
    wfct_d = nc.dram_tensor("wfct", [128, 512], f16, kind="ExternalInput").ap()
    bias16_d = nc.dram_tensor("bias16", [16, 128], f16, kind="ExternalInput").ap()
    ind_d = nc.dram_tensor("ind", [16, 512], f16, kind="ExternalInput").ap()
    i32sel_d = nc.dram_tensor("i32sel", [128, 128], f16, kind="ExternalInput").ap()
    mbqb_d = nc.dram_tensor("mbqb", [128, 512], f16, kind="ExternalInput").ap()
    bfc1_d = nc.dram_tensor("bfc1", [1, 128], f16, kind="ExternalInput").ap()
    o_d = nc.dram_tensor("o", [O, BB], f32, kind="ExternalOutput").ap()

    with tile.TileContext(nc) as tc, ExitStack() as ctx:
        wpool = ctx.enter_context(tc.tile_pool(name="wpool", bufs=1))
        stp = ctx.enter_context(tc.tile_pool(name="stp", bufs=2))
        state = ctx.enter_context(tc.tile_pool(name="state", bufs=2))
        pers = ctx.enter_context(tc.tile_pool(name="pers", bufs=1))
        attn_ps = ctx.enter_context(tc.tile_pool(name="attn_ps", bufs=2, space="PSUM"))
        gi_ps = ctx.enter_context(tc.tile_pool(name="gi_ps", bufs=2, space="PSUM"))
        go_ps = ctx.enter_context(tc.tile_pool(name="go_ps", bufs=2, space="PSUM"))
        pre_ps = ctx.enter_context(tc.tile_pool(name="pre_ps", bufs=2, space="PSUM"))

        # ---- static weights / inputs into SBUF ----
        def wload(name, shape, dt_, src):
            t_ = wpool.tile(shape, dt_, tag=name)
            nc.sync.dma_start(t_[:], src[:])
            return t_
        Z = wload("Z", [128, TB], f16, z_d)
        ZP = wload("ZP", [128, TB], f16, zp_d)
        MW = wload("MW", [128, 16 * 128], f16, mw_d)
        WHH = wload("WHH", [128, 64 * 128], f16, whh_d)
        WIH = wload("WIH", [128, 16 * 128], f16, wih_d)
        MEMBF = wload("MEMBF", [128, 512], bf16, membf_d)
        RZ = wload("RZ", [128, 512], f16, rz_d)
        RZP = wload("RZP", [128, 512], f16, rzp_d)
        WFCT = wload("WFCT", [128, 512], f16, wfct_d)
        BIAS16 = wload("BIAS16", [16, 128], f16, bias16_d)
        IND = wload("IND", [16, 512], f16, ind_d)
        I32S = wload("I32S", [128, 128], f16, i32sel_d)
        MBQB = wload("MBQB", [128, 512], f16, mbqb_d)
        BFC1 = wload("BFC1", [1, 128], f16, bfc1_d)
        ONESB = wpool.tile([128, 128], bf16, tag="ONESB")
        nc.vector.memset(ONESB[:], 1.0)
        ONES32 = wpool.tile([1, 32], f16, tag="ONES32")
        nc.vector.memset(ONES32[:], 1.0)

        logz = wpool.tile([128, NSL * 512], f16, tag="logz")

        HT = pers.tile([128, 128], f16, tag="HT")
        Dt = pers.tile([128, 128], f16, tag="Dt")
        nc.vector.memset(HT[:], 0.0)
        nc.vector.memset(Dt[:], 0.0)

        # logz slice s: [tb-part, m] = Z_s.T @ RZ + ZP_s.T @ RZP, + mbq via
        # a Pool-engine add during the PSUM->SBUF copy (Pool is otherwise idle).
        def logz_slice(s):
            pp = pre_ps.tile([128, 512], f32, tag="pp")
            nc.tensor.matmul(pp[:], lhsT=Z[:, 128 * s:128 * (s + 1)],
                             rhs=RZ[:], start=True, stop=False)
            nc.tensor.matmul(pp[:], lhsT=ZP[:, 128 * s:128 * (s + 1)],
                             rhs=RZP[:], start=False, stop=True)
            # (GPSIMD cannot access PSUM; V is idle at step start anyway)
            nc.vector.tensor_tensor(logz[:, 512 * s:512 * (s + 1)], pp[:],
                                    MBQB[:], ALU.add)

        with nc.named_scope("precompute"):
            for s in range(min(4, NSL)):
                logz_slice(s)

        # ---- recurrence ----
        for t in range(T):
            tg, tl = t // 4, t % 4
            with nc.named_scope(f"step{t}" if t % 10 == 0 else "step"):
                if 4 + t < NSL:              # stream remaining logz slices
                    logz_slice(4 + t)        # (ready long before step 4*(4+t))
                pa = attn_ps.tile([128, 512], f32, tag="pa")
                pgi = gi_ps.tile([128, 384], f32, tag="pgi")
                pgo = go_ps.tile([128, 128], f32, tag="pgo")

                # gate bias into psum (constants only; runs during prev tail)
                nc.tensor.matmul(pgi[:], lhsT=BIAS16[:], rhs=IND[:, 0:384],
                                 start=True, stop=False)
                nc.tensor.matmul(pgo[:], lhsT=BIAS16[:], rhs=IND[:, 384:512],
                                 start=True, stop=False)
                # logits: one sequential accumulation group per 32-col slice
                for j in range(4):
                    nc.tensor.matmul(pa[:, 32 * j:32 * (j + 1)],
                                     lhsT=logz[:, 512 * tg + 128 * j:512 * tg + 128 * (j + 1)],
                                     rhs=I32S[:, 32 * tl:32 * tl + 32],
                                     start=True, stop=False)
                    for k in range(4):
                        nc.tensor.matmul(pa[:, 32 * j:32 * (j + 1)],
                                         lhsT=MW[:, 128 * (4 * j + k):128 * (4 * j + k + 1)],
                                         rhs=HT[:, 32 * k:32 * k + 32],
                                         start=False, stop=(k == 3))
                # e^T = exp(logits^T)  [128 m-part, (chunk, b)]
                ET = stp.tile([128, 128], bf16, tag="ET")
                nc.scalar.activation(ET[:], pa[:, 0:128], AF.Exp)

                def whh_block(glo, ghi):
                    for g in range(glo, ghi):
                        dst = pgi if g < 12 else pgo
                        off = 32 * g if g < 12 else 32 * (g - 12)
                        for k in range(4):
                            nc.tensor.matmul(dst[:, off:off + 32],
                                             lhsT=WHH[:, 128 * (4 * g + k):128 * (4 * g + k + 1)],
                                             rhs=HT[:, 32 * k:32 * k + 32],
                                             start=False, stop=False)

                # all W_hh MMs for i,f,g chunks BEFORE sums/gd: the [sums, gd]
                # batch then sits between two wait boundaries so its semaphore
                # increment fires right at gd completion (early recip signal).
                whh_block(0, 12)      # 48 MMs (exp runs concurrently)
                # Priorities: the scheduler orders by priority respecting deps.
                # Place sums/gd ~16 whh-MMs into the stream (covers exp
                # latency) and ih ~24 whh-MMs later (covers recip->gdn), so
                # the stalls overlap W_hh work instead of serializing.
                with tc.high_priority(offset=40):
                    # sums bcast over partitions: pa[:,128:160] = sum_m e
                    for c in range(4):
                        nc.tensor.matmul(pa[:, 128:160], lhsT=ONESB[:],
                                         rhs=ET[:, 32 * c:32 * c + 32],
                                         start=(c == 0), stop=(c == 3))
                    # gd^T = memory^T-chunks @ e^T
                    for j in range(4):
                        nc.tensor.matmul(pa[:, 160:192],
                                         lhsT=MEMBF[:, 128 * j:128 * (j + 1)],
                                         rhs=ET[:, 32 * j:32 * j + 32],
                                         start=(j == 0), stop=(j == 3))
                    recipB = stp.tile([128, 32], f32, tag="recipB")
                    with nc.allow_low_precision(reason="softmax recip approx ok"):
                        nc.vector.reciprocal_approx_fast(recipB[:], pa[:, 128:160])
                    GDN = stp.tile([128, 32], f16, tag="GDN")
                    nc.vector.tensor_tensor(GDN[:], pa[:, 160:192], recipB[:],
                                            ALU.mult)
                with tc.high_priority(offset=16):
                    # W_ih @ gdn for i, f, g chunks
                    for g in range(12):
                        nc.tensor.matmul(pgi[:, 32 * g:32 * (g + 1)],
                                         lhsT=WIH[:, 128 * g:128 * (g + 1)],
                                         rhs=GDN[:], start=False, stop=(g == 11))
                whh_block(12, 16)     # o chunks while tanh/c-chain run
                for g in range(12, 16):
                    nc.tensor.matmul(pgo[:, 32 * (g - 12):32 * (g - 11)],
                                     lhsT=WIH[:, 128 * g:128 * (g + 1)],
                                     rhs=GDN[:], start=False, stop=(g == 15))

                # pointwise: Y = tanh(gates_scaled), fp16
                Yifg = stp.tile([128, 384], f16, tag="Yifg")
                nc.scalar.activation(Yifg[:], pgi[:], AF.Tanh)
                Yo = stp.tile([128, 128], f16, tag="Yo")
                nc.scalar.activation(Yo[:], pgo[:], AF.Tanh)
                m1 = stp.tile([128, 128], f16, tag="m1")
                nc.vector.scalar_tensor_tensor(m1[:], Yifg[:, 128:256], 1.0, Dt[:],
                                               ALU.add, ALU.mult)
                m2 = stp.tile([128, 128], f16, tag="m2")
                nc.vector.scalar_tensor_tensor(m2[:], Yifg[:, 0:128], 1.0,
                                               Yifg[:, 256:384], ALU.add, ALU.mult)
                Dn = state.tile([128, 128], f16, tag="Dn")
                nc.vector.scalar_tensor_tensor(Dn[:], m1[:], 0.5, m2[:],
                                               ALU.mult, ALU.add)
                TC = stp.tile([128, 128], f16, tag="TC")
                nc.scalar.activation(TC[:], Dn[:], AF.Tanh, scale=0.5)
                Hn = state.tile([128, 128], f16, tag="Hn")
                nc.vector.scalar_tensor_tensor(Hn[:], Yo[:], 1.0, TC[:],
                                               ALU.add, ALU.mult)
                HT, Dt = Hn, Dn

        # ---- final output: out^T = 0.5*W_fc @ H + b_fc ----
        with nc.named_scope("final"):
            pf = attn_ps.tile([128, 512], f32, tag="pa")
            nc.tensor.matmul(pf[:, 0:32], lhsT=BFC1[:], rhs=ONES32[:],
                             start=True, stop=False)
            for k in range(4):
                nc.tensor.matmul(pf[:, 0:32], lhsT=WFCT[:, 128 * k:128 * (k + 1)],
                                 rhs=HT[:, 32 * k:32 * k + 32],
                                 start=False, stop=(k == 3))
            outt = stp.tile([O, BB], f32, tag="outt")
            nc.scalar.activation(outt[:], pf[:, 0:32], AF.Identity)
            nc.sync.dma_start(o_d[:], outt[:])

    nc.compile()
    return nc


def _prep_host(inputs):
    """Host-side: fold weights, compute z/zp gating streams, build in_maps."""
    f16 = np.float16
    import ml_dtypes
    b16 = ml_dtypes.bfloat16
    inp = {k: np.asarray(v, np.float32) for k, v in inputs.items()}
    mem = inp["memory"]
    Wq = inp["W_q"]
    Wq1, Wq2, Wq3 = Wq[:, :F], Wq[:, F:2 * F], Wq[:, 2 * F:]

    # gate row scaling: 0.5 for i,f,o (tanh trick), 1.0 for g; W_hh also *0.5 (H=2h)
    r = np.full((4 * H, 1), 0.5, np.float32)
    r[2 * H:3 * H] = 1.0
    WIHs = r * inp["W_ih"]
    WHHs = r * inp["W_hh"] * 0.5
    biass = r[:, 0] * (inp["b_ih"] + inp["b_hh"])

    wih = np.empty((128, 16 * 128), np.float32)
    for g in range(16):
        wih[:, 128 * g:128 * (g + 1)] = WIHs[128 * g:128 * (g + 1), :].T
    whh = np.empty((128, 64 * 128), np.float32)
    for g in range(16):
        for k in range(4):
            whh[:, 128 * (4 * g + k):128 * (4 * g + k + 1)] = \
                WHHs[128 * g:128 * (g + 1), 128 * k:128 * (k + 1)].T
    MWmat = 0.5 * (mem @ (Wq3 @ inp["W_fc"]))          # [M, H]
    mw = np.empty((128, 16 * 128), np.float32)
    for j in range(4):
        for k in range(4):
            mw[:, 128 * (4 * j + k):128 * (4 * j + k + 1)] = \
                MWmat[128 * j:128 * (j + 1), 128 * k:128 * (k + 1)].T
    membf = np.empty((128, 512), np.float32)
    for j in range(4):
        membf[:, 128 * j:128 * (j + 1)] = mem[128 * j:128 * (j + 1), :]
    wfct = np.empty((128, 512), np.float32)
    WFCs = (0.5 * inp["W_fc"]).T                       # [H, O]
    for k in range(4):
        wfct[:, 128 * k:128 * (k + 1)] = WFCs[128 * k:128 * (k + 1), :]

    bias16 = biass.reshape(16, 128)
    ind = np.zeros((16, 512), np.float32)
    for g in range(16):
        ind[g, 32 * g:32 * (g + 1)] = 1.0
    i32sel = np.eye(128, dtype=f16)
    mbq = mem @ (inp["b_q"] + Wq3 @ inp["b_fc"])       # [M]
    mbqb = np.broadcast_to(mbq[None, :], (128, 512)).copy()
    bfc1 = inp["b_fc"][None, :]

    shared = dict(
        mw=mw.astype(f16), whh=whh.astype(f16), wih=wih.astype(f16),
        membf=membf.astype(b16), rz=(mem @ Wq1).T.astype(f16),
        rzp=(mem @ Wq2).T.astype(f16), wfct=wfct.astype(f16),
        bias16=bias16.astype(f16), ind=ind.astype(f16), i32sel=i32sel,
        mbqb=mbqb.astype(f16), bfc1=bfc1.astype(f16),
    )

    # z/zp gating streams on host (input-only elementwise preprocessing)
    x = inp["input"]                                   # [B, 6, 100, F]
    X, Xl, Mask, Delta, Xlb, Dltb = (x[:, i, :T] for i in range(6))
    Xm = inp["X_mean"][None, :T, :]                    # [1, T, F]
    dgz = np.diag(inp["W_gz"])[None, None, :]
    bgz = inp["b_gz"][None, None, :]
    dgzp = np.diag(inp["W_gzp"])[None, None, :]
    bgzp = inp["b_gzp"][None, None, :]
    dz = np.minimum(np.exp(-dgz * Delta - bgz), 1.0)
    dzp = np.minimum(np.exp(-dgzp * Dltb - bgzp), 1.0)
    zfull = Mask * X + (1 - Mask) * (dz * Xl + (1 - dz) * Xm)      # [B, T, F]
    zpfull = Mask * X + (1 - Mask) * (dzp * Xlb + (1 - dzp) * Xm)

    in_maps = []
    for core in range(NC):
        b0 = core * BB
        m_ = dict(shared)
        # [BB, T, F] -> [F, T*BB]
        m_["z"] = np.ascontiguousarray(
            np.transpose(zfull[b0:b0 + BB], (2, 1, 0)).reshape(F, TB)).astype(f16)
        m_["zp"] = np.ascontiguousarray(
            np.transpose(zpfull[b0:b0 + BB], (2, 1, 0)).reshape(F, TB)).astype(f16)
        in_maps.append(m_)
    return in_maps


def kernel(**inputs):
    global _built
    from concourse import bass_utils
    if _built is None:
        _built = _build()
    in_maps = _prep_host(inputs)
    res = bass_utils.run_bass_kernel_spmd(_built, in_maps, core_ids=list(range(NC)))
    out = np.empty((B, 1, O), np.float32)
    for core in range(NC):
        out[core * BB:(core + 1) * BB, 0, :] = res.results[core]["o"].T
    return out
